# revision 15
# baseline (speedup 1.0000x reference)
"""CAM-module (channel attention) Trainium2 kernel.

Problem: B=4 samples, C=64, H=W=256 (N=65536 px). concat(rgb,hsv,lab) ->
X [192, N] per sample; q/k/v = 1x1-conv projections (W [64,192] + bias);
energy = q @ k^T * C^-0.5 -> softmax over last dim -> out = att @ v.

Sharding: 8 cores = 4 samples x 2 spatial halves (32768 px each). Each
core computes a partial energy over its half; a 16 KiB pairwise AllReduce
([[0,1],[2,3],[4,5],[6,7]]) completes the C x C energy, then each core
computes out for its own half.

Precision: X is cast to fp16 on the HOST (halves input DMA: 12.6 MB/core).
Dropping X's fp16 residual gives q/k elem error ~1.5e-4 -> energy logit
error ~0.04 absolute vs logit spread ~850 - verified against the exact
(deterministic) reference data. W keeps an fp16 hi/lo split so the
projection is X_h @ W exact to ~2^-22: 2 matmul passes per 128-channel
chunk. Energy accumulates in true fp32 on PE. v = Wvh @ Xh in one fp16
pass; out = fp16(att) @ fp16(v); output DMAs back as fp16 (halves output
traffic), host casts to fp32.

Biases fold in exactly via an appended ones-row on X (lab chunk becomes
65 partitions) and bias rows on the weight chunks.
"""

import sys
import numpy as np

if '/opt/trn_rl_repo' not in sys.path:
    sys.path.insert(0, '/opt/trn_rl_repo')

B, C, H, W = 4, 64, 256, 256
N = H * W                 # 65536 px per sample
NHALF = N // 2            # 32768 px per core
PX = 2048                 # streaming tile (px)
NIT = NHALF // PX         # 16
SUB = 128                 # qkT subtile (px) = matmul M
NSUB = PX // SUB          # 16
VC = 512                  # v / out chunk (px) = matmul N
NVC = PX // VC            # 4
NCORES = 8

_CACHE = {}


def _build_bass(single_core=False):
    import concourse.bacc as bacc
    import concourse.mybir as mybir
    from concourse import tile

    F32 = mybir.dt.float32
    F16 = mybir.dt.float16
    Exp = mybir.ActivationFunctionType.Exp

    nc = bacc.Bacc("TRN2", target_bir_lowering=False, debug=False,
                   enable_asserts=False,
                   num_devices=1 if single_core else NCORES)

    x0_d = nc.dram_tensor("x0", [128, NHALF], F16, kind="ExternalInput").ap()
    x1_d = nc.dram_tensor("x1", [64, NHALF], F16, kind="ExternalInput").ap()
    # packed weights: cols [wqkh 0:128 | wqkl 128:256 | wvh 256:320]
    w0_d = nc.dram_tensor("w0", [128, 320], F16, kind="ExternalInput").ap()
    w1_d = nc.dram_tensor("w1", [65, 320], F16, kind="ExternalInput").ap()
    ident_d = nc.dram_tensor("ident", [64, 64], F32, kind="ExternalInput").ap()
    out_d = nc.dram_tensor("out", [64, NHALF], F16, kind="ExternalOutput").ap()

    with tile.TileContext(nc) as tc:
        with tc.tile_pool(name="const", bufs=1) as const, \
             tc.tile_pool(name="stream", bufs=5) as stream, \
             tc.tile_pool(name="qk", bufs=3) as qkpool, \
             tc.tile_pool(name="outp", bufs=4) as outp, \
             tc.tile_pool(name="qkps", bufs=2, space="PSUM") as qkps, \
             tc.tile_pool(name="vps", bufs=4, space="PSUM") as vps, \
             tc.tile_pool(name="eps", bufs=1, space="PSUM") as eps, \
             tc.tile_pool(name="dram", bufs=1, space="DRAM") as dram:

            w0 = const.tile([128, 320], F16)
            w1 = const.tile([65, 320], F16)
            ident = const.tile([64, 64], F32)
            nc.scalar.dma_start(w0[:], w0_d[:])
            nc.scalar.dma_start(w1[:], w1_d[:])
            nc.scalar.dma_start(ident[:], ident_d[:])
            wqkh0, wqkl0, wvh0 = w0[:, 0:128], w0[:, 128:256], w0[:, 256:320]
            wqkh1, wqkl1, wvh1 = w1[:, 0:128], w1[:, 128:256], w1[:, 256:320]

            # preload the ACT Exp table set off the critical path (~2.7us)
            warm = const.tile([1, 1], F32)
            nc.gpsimd.memset(warm[:], 0.0)
            nc.scalar.activation(warm[:], warm[:], Exp)

            vh = const.tile([64, NHALF], F16)     # v values, fp16
            ep = eps.tile([64, 64], F32)          # energy accumulator

            for it in range(NIT):
                sl = slice(it * PX, (it + 1) * PX)
                x0h = stream.tile([128, PX], F16, tag="x0h")
                nc.sync.dma_start(x0h[:], x0_d[:, sl])
                x1h = stream.tile([65, PX], F16, tag="x1h")
                nc.sync.dma_start(x1h[0:64, :], x1_d[:, sl])
                if it < 5:
                    # ones rows live in the 5 round-robin pool slots;
                    # later iterations reuse them untouched
                    nc.gpsimd.memset(x1h[64:65, :], 1.0)

                for grp in range(NSUB // 4):   # qkT: 4 subtiles per PSUM bank
                    qkp = qkps.tile([128, 512], F32, tag="qkp")
                    for s4 in range(4):
                        sb = grp * 4 + s4
                        ssl = slice(sb * SUB, (sb + 1) * SUB)
                        osl = slice(s4 * 128, (s4 + 1) * 128)
                        nc.tensor.matmul(qkp[:, osl], x0h[:, ssl], wqkh0[:], start=True, stop=False)
                        nc.tensor.matmul(qkp[:, osl], x0h[:, ssl], wqkl0[:], start=False, stop=False)
                        nc.tensor.matmul(qkp[:, osl], x1h[:, ssl], wqkh1[:], start=False, stop=False)
                        nc.tensor.matmul(qkp[:, osl], x1h[:, ssl], wqkl1[:], start=False, stop=True)
                    qk_sb = qkpool.tile([128, 512], F32, tag="qk_sb")
                    nc.scalar.copy(qk_sb[:], qkp[:])
                    for s4 in range(4):
                        first = (it == 0 and grp == 0 and s4 == 0)
                        last = (it == NIT - 1 and grp == NSUB // 4 - 1 and s4 == 3)
                        nc.tensor.matmul(ep[:], qk_sb[:, s4 * 128:s4 * 128 + 64],
                                         qk_sb[:, s4 * 128 + 64:s4 * 128 + 128],
                                         start=first, stop=last)

                # v for an EARLIER tile (1 pass, Ldweights amortized over the
                # 4 chunks): delaying v by 2 iterations leaves PE v-work to
                # chew on while the AllReduce runs after the final E matmul
                def v_block(vit, vx0h, vx1h):
                    # 4 [64,512] PSUM chunks; wvh0 stays loaded across the 4
                    # first-pass matmuls, wvh1 across the second pass
                    vts = []
                    for _c in range(NVC):
                        vpc = vps.tile([64, VC], F32, tag="vp")
                        vts.append(vpc)
                    for c in range(NVC):
                        vsl = slice(c * VC, (c + 1) * VC)
                        nc.tensor.matmul(vts[c][:], wvh0[:], vx0h[:, vsl],
                                         start=True, stop=False)
                    for c in range(NVC):
                        vsl = slice(c * VC, (c + 1) * VC)
                        nc.tensor.matmul(vts[c][:], wvh1[:], vx1h[:, vsl],
                                         start=False, stop=True)
                    for c in range(NVC):
                        gsl = slice(vit * PX + c * VC, vit * PX + (c + 1) * VC)
                        if c % 2 == 0:
                            nc.vector.tensor_copy(vh[:, gsl], vts[c][:])
                        else:
                            nc.scalar.copy(vh[:, gsl], vts[c][:])

                if it > 3:
                    v_block(it - 4, *pending.pop(0))
                pending = (pending if it > 0 else []) + [(x0h, x1h)]

            for j in range(4):
                v_block(NIT - 4 + j, *pending[j])

            # partial energy -> pairwise AllReduce
            e_sb = const.tile([64, 64], F32)
            nc.scalar.copy(e_sb[:], ep[:])
            bi = dram.tile([64, 64], F32)
            bo = dram.tile([64, 64], F32)
            nc.sync.dma_start(bi[:], e_sb[:])
            if single_core:
                nc.gpsimd.dma_start(bo[:], bi[:])
            else:
                nc.gpsimd.collective_compute(
                    "AllReduce", mybir.AluOpType.add,
                    replica_groups=[[0, 1], [2, 3], [4, 5], [6, 7]],
                    ins=[bi.opt()], outs=[bo.opt()],
                )
            e2 = const.tile([64, 64], F32)
            nc.sync.dma_start(e2[:], bo[:])

            # softmax over free dim, scale C^-0.5 = 0.125 folded into exp
            m = const.tile([64, 1], F32)
            nc.vector.reduce_max(m[:], e2[:], axis=mybir.AxisListType.X)
            mb = const.tile([64, 1], F32)
            nc.vector.tensor_scalar_mul(mb[:], m[:], -0.125)
            attu = const.tile([64, 64], F32)
            s = const.tile([64, 1], F32)
            nc.scalar.activation(attu[:], e2[:], Exp, bias=mb[:], scale=0.125,
                                 accum_out=s[:])
            r = const.tile([64, 1], F32)
            nc.vector.reciprocal(r[:], s[:])
            att = const.tile([64, 64], F32)
            nc.vector.tensor_scalar_mul(att[:], attu[:], r[:])

            # att^T (PE transpose), cast fp16
            atp = vps.tile([64, 64], F32, tag="vp")
            nc.tensor.transpose(atp[:], att[:], ident[:])
            attT = const.tile([64, 64], F16)
            nc.scalar.copy(attT[:], atp[:])

            # out = att @ vh, [64,512] PSUM chunks 4-deep; attT stays loaded
            # in the PE array (one Ldweights), copies rotate ACT/DVE/Pool,
            # DMA per 2048px
            for g in range(NHALF // PX):
                out_sb = outp.tile([64, PX], F16, tag="out_sb")
                for c in range(NVC):
                    op = vps.tile([64, VC], F32, tag="vp")
                    gsl = slice(g * PX + c * VC, g * PX + (c + 1) * VC)
                    nc.tensor.matmul(op[:], attT[:], vh[:, gsl],
                                     start=True, stop=True)
                    osl = slice(c * VC, (c + 1) * VC)
                    if c % 2 == 0:
                        nc.scalar.copy(out_sb[:, osl], op[:])
                    else:
                        nc.vector.tensor_copy(out_sb[:, osl], op[:])
                nc.sync.dma_start(out_d[:, g * PX:(g + 1) * PX], out_sb[:])

    nc.compile()
    return nc


def _get_nc():
    if 'nc' not in _CACHE:
        _CACHE['nc'] = _build_bass()
    return _CACHE['nc']


def _split16(a):
    h = a.astype(np.float16)
    l = (a - h.astype(np.float32)).astype(np.float16)
    return h, l


def kernel(rgb, hsv, lab, Wq, bq, Wk, bk, Wv, bv):
    from concourse.bass_utils import run_bass_kernel_spmd

    nc = _get_nc()

    rgb = np.asarray(rgb, dtype=np.float32)
    hsv = np.asarray(hsv, dtype=np.float32)
    lab = np.asarray(lab, dtype=np.float32)
    Wq = np.asarray(Wq, dtype=np.float32)
    Wk = np.asarray(Wk, dtype=np.float32)
    Wv = np.asarray(Wv, dtype=np.float32)
    bq = np.asarray(bq, dtype=np.float32)
    bk = np.asarray(bk, dtype=np.float32)
    bv = np.asarray(bv, dtype=np.float32)

    # weight prep: [192ch + ones-row, outs] with bias row, fp16 hi/lo
    wqk = np.concatenate([Wq.T, Wk.T], axis=1)          # [192, 128]
    bqk = np.concatenate([bq, bk])                      # [128]
    wqk_aug = np.vstack([wqk, bqk[None, :]])            # [193, 128]
    wqkh, wqkl = _split16(wqk_aug)
    wv_aug = np.vstack([Wv.T, bv[None, :]])             # [193, 64]
    wvh, _ = _split16(wv_aug)

    shared = {
        "w0": np.ascontiguousarray(
            np.concatenate([wqkh[0:128], wqkl[0:128], wvh[0:128]], axis=1)),
        "w1": np.ascontiguousarray(
            np.concatenate([wqkh[128:193], wqkl[128:193], wvh[128:193]], axis=1)),
        "ident": np.eye(64, dtype=np.float32),
    }

    in_maps = []
    for c in range(NCORES):
        b, half = c // 2, c % 2
        hs = slice(half * (H // 2), (half + 1) * (H // 2))
        x0 = np.empty((128, NHALF), dtype=np.float16)
        x0[0:64] = rgb[b, :, hs, :].reshape(C, NHALF)
        x0[64:128] = hsv[b, :, hs, :].reshape(C, NHALF)
        in_maps.append({
            "x0": x0,
            "x1": np.ascontiguousarray(
                lab[b, :, hs, :].reshape(C, NHALF).astype(np.float16)),
            **shared,
        })

    res = run_bass_kernel_spmd(nc, in_maps, core_ids=list(range(NCORES)),
                               **_CACHE.get('run_kwargs', {}))
    _CACHE['last_results'] = res
    _CACHE['last_in_maps'] = in_maps

    out = np.empty((B, C, H, W), dtype=np.float32)
    for c in range(NCORES):
        b, half = c // 2, c % 2
        hs = slice(half * (H // 2), (half + 1) * (H // 2))
        out[b, :, hs, :] = res.results[c]["out"].astype(np.float32).reshape(C, H // 2, W)
    return out
